# revision 27
# baseline (speedup 1.0000x reference)
# GNN message-passing (GENConv-style, 3 layers x 2 link types) on 8 TRN2 cores.
#
# Strategy (node-sharded, gather-from-replicated-tables):
#   - Each core owns a contiguous shard of SH = N/R nodes.
#   - Host precomposes per-(node,k) indices: a = ei0[nbr], b = ei1[nbr], c = ea[nbr].
#   - Per layer-link, three gather passes over the shard's [SH, K, H] workload:
#       P1: gather src rows by a, msg = relu(src_a + c*we); m = max_k msg  (cache msg)
#       P2: gather (-m) rows by b, q = exp(msg - m_b); s = sum_k q; u = msg*q (cache u)
#       P3: gather (1/s) rows by b, agg = sum_k u * rs_b
#     Tables (src=h, -m, 1/(s+1e-16)) live node-major in DRAM; shards are
#     all-gathered across the 8 cores between passes.
#   - MLP tail per link runs transposed on PE: h1_T = w1T.T@(aggT + dstT),
#     BN+relu fused into one ACT op (per-partition scale/bias), y += w2T.T@relu1.
#   - Layer-0 input projections (x@wsT / x@wdT@w1T) are host-folded into the
#     src0 gather table and a d1T additive term.
#
# The kernel() entry point takes the FULL unsharded inputs (as produced by
# reference.setup_inputs) and returns the FULL [N, H] output.

import os
from contextlib import ExitStack

import numpy as np

import concourse.bass as bass
import concourse.mybir as mybir
import concourse.tile as tile
from concourse import bacc
from concourse.bass import IndirectOffsetOnAxis
from concourse.masks import make_identity

F32 = mybir.dt.float32
I32 = mybir.dt.int32
AF = mybir.ActivationFunctionType
ALU = mybir.AluOpType

P = 128  # partitions


def full_cfg():
    return dict(N=16384, K=16, L=2, NL=3, CIN=170, H=128, R=8)


def _derived(cfg):
    N, K, L, NL, H, R = cfg["N"], cfg["K"], cfg["L"], cfg["NL"], cfg["H"], cfg["R"]
    SH = N // R
    assert SH % P == 0
    T = SH // P
    FB = K * H  # free width of one node-tile in the cache
    C2 = 2 * H
    CH = 512  # matmul free-dim chunk
    assert SH % CH == 0
    NCH = SH // CH
    return SH, T, FB, C2, CH, NCH


def build_program(cfg, nc=None):
    """Builds the SPMD per-core Bass program standalone. Returns nc."""
    N, K, L, NL, H, R = cfg["N"], cfg["K"], cfg["L"], cfg["NL"], cfg["H"], cfg["R"]
    SH, T, FB, C2, CH, NCH = _derived(cfg)

    if nc is None:
        nc = bacc.Bacc("TRN2", num_devices=R, debug=False)

    ins = dict(
        src0=nc.dram_tensor("src0", [L * N, H], F32, kind="ExternalInput").ap(),
        d1T=nc.dram_tensor("d1T", [L * C2, SH], F32, kind="ExternalInput").ap(),
        aidx=nc.dram_tensor("aidx", [P, L * T * K], I32, kind="ExternalInput").ap(),
        bidx=nc.dram_tensor("bidx", [P, L * T * K], I32, kind="ExternalInput").ap(),
        cnm=nc.dram_tensor("cnm", [P, L * T * K], F32, kind="ExternalInput").ap(),
        web=nc.dram_tensor("web", [P, NL * L * H], F32, kind="ExternalInput").ap(),
        w1T=nc.dram_tensor("w1T", [P, NL * L * C2], F32, kind="ExternalInput").ap(),
        w2T=nc.dram_tensor("w2T", [P, NL * L * 2 * H], F32,
                           kind="ExternalInput").ap(),
        bnA=nc.dram_tensor("bnA", [P, NL * L * 2], F32, kind="ExternalInput").ap(),
        bnB=nc.dram_tensor("bnB", [P, NL * L * 2], F32, kind="ExternalInput").ap(),
    )
    outs = dict(out=nc.dram_tensor("out", [SH, H], F32, kind="ExternalOutput").ap())

    with tile.TileContext(nc) as tc:
        build_body(tc, outs, ins, cfg)
    if isinstance(nc, bacc.Bacc):
        nc.compile()
    return nc


def build_body(tc, outs, ins, cfg):
    """Emit the kernel body into TileContext tc. outs/ins: dicts of DRAM APs."""
    nc = tc.nc
    N, K, L, NL, H, R = cfg["N"], cfg["K"], cfg["L"], cfg["NL"], cfg["H"], cfg["R"]
    SH, T, FB, C2, CH, NCH = _derived(cfg)

    src0_d = ins["src0"]
    d1T_d = ins["d1T"]
    aidx_d = ins["aidx"]
    bidx_d = ins["bidx"]
    cnm_d = ins["cnm"]
    web_d = ins["web"]
    w1T_d = ins["w1T"]
    w2T_d = ins["w2T"]
    bnA_d = ins["bnA"]
    bnB_d = ins["bnB"]
    out_d = outs["out"]

    rg = [list(range(R))]
    assert T % 2 == 0
    HT = T // 2  # tiles per half-shard (AG granularity)
    # Shared collective outputs need >4 cores; fall back to Local for small sims
    shared_as = "Shared" if R > 4 else "Local"

    with ExitStack() as ctx:
        constp = ctx.enter_context(tc.tile_pool(name="constp", bufs=1))
        cachep = ctx.enter_context(tc.tile_pool(name="cachep", bufs=1))
        gbufp = ctx.enter_context(tc.tile_pool(name="gbufp", bufs=4))
        statsp = ctx.enter_context(tc.tile_pool(name="statsp", bufs=1))
        smallp = ctx.enter_context(tc.tile_pool(name="smallp", bufs=2))
        aggp = ctx.enter_context(tc.tile_pool(name="aggp", bufs=1))
        htp = ctx.enter_context(tc.tile_pool(name="htp", bufs=1))
        ysp = ctx.enter_context(tc.tile_pool(name="ysp", bufs=1, space="PSUM"))
        bigcp = ctx.enter_context(tc.tile_pool(name="bigcp", bufs=1))
        dramp = ctx.enter_context(tc.tile_pool(name="dramp", bufs=1, space="DRAM"))

        # ---- persistent constants ----
        ident = constp.tile([P, P], F32, name="ident")
        make_identity(nc, ident[:])
        bnA_s = constp.tile([P, NL * L * 2], F32, name="bnA_s")
        nc.sync.dma_start(out=bnA_s[:], in_=bnA_d)
        bnB_s = constp.tile([P, NL * L * 2], F32, name="bnB_s")
        nc.sync.dma_start(out=bnB_s[:], in_=bnB_d)
        web_s = constp.tile([P, NL * L * H], F32, name="web_s")
        nc.sync.dma_start(out=web_s[:], in_=web_d)
        cnm_s = constp.tile([P, L * T * K], F32, name="cnm_s")
        nc.sync.dma_start(out=cnm_s[:], in_=cnm_d)
        aidx_s = constp.tile([P, L * T * K], I32, name="aidx_s")
        nc.sync.dma_start(out=aidx_s[:], in_=aidx_d)
        bidx_s = constp.tile([P, L * T * K], I32, name="bidx_s")
        nc.sync.dma_start(out=bidx_s[:], in_=bidx_d)

        cache = cachep.tile([P, T * FB], F32, name="cache")

        h_T_prev = None
        h_full_prev = None

        for i in range(NL):
            y_ps = ysp.tile([P, SH], F32, name=f"y_{i}", tag="y")
            for l in range(L):
                il = i * L + l
                if i == 0:
                    src_tab = src0_d
                    src_elem_off = l * N * H
                else:
                    src_tab = h_full_prev[:]
                    src_elem_off = 0

                # ---------------- PASS 1 ----------------
                # cache_t = c*we; gather-ADD src rows (CCE, exact IEEE add);
                # msg = max(.,0)+1e-7 (DVE max kills NaN like XLA's maximum);
                # mneg = -max_k msg
                mneg_sh = statsp.tile([P, T * H], F32, name=f"st_{il}a", tag="stats")
                mneg_full = dramp.tile([N, H], F32, name=f"mneg_full_{il}",
                                       addr_space=shared_as if R > 1 else "Local")
                if True:
                    for t in range(T):
                        ct = cache[:, t * FB:(t + 1) * FB]
                        # cwe = c[:,k] * we  (ACT broadcast-mul, per-k)
                        for k in range(K):
                            col = (l * T + t) * K + k
                            nc.scalar.mul(
                                ct[:, k * H:(k + 1) * H],
                                web_s[:, il * H:(il + 1) * H],
                                mul=cnm_s[:, col:col + 1],
                            )
                        nc.gpsimd.indirect_dma_start(
                            out=ct,
                            out_offset=None,
                            in_=src_tab,
                            in_offset=IndirectOffsetOnAxis(
                                ap=aidx_s[:, (l * T + t) * K:(l * T + t + 1) * K],
                                axis=0,
                            ),
                            element_offset=src_elem_off,
                            compute_op=ALU.add,
                        )
                        nc.vector.tensor_scalar(
                            out=ct, in0=ct, scalar1=0.0, scalar2=1e-7,
                            op0=ALU.max, op1=ALU.add)
                        nc.vector.tensor_reduce(
                            out=mneg_sh[:, t * H:(t + 1) * H],
                            in_=ct.rearrange("p (k h) -> p h k", k=K),
                            axis=mybir.AxisListType.X, op=ALU.max, negate=True)

                    mneg_loc = dramp.tile([SH, H], F32, name=f"mneg_loc_{il}")
                    nc.sync.dma_start(
                        out=mneg_loc[:].rearrange("(t p) h -> p t h", p=P),
                        in_=mneg_sh[:].rearrange("p (t h) -> p t h", t=T))
                    if R > 1:
                        nc.gpsimd.collective_compute(
                            "AllGather", ALU.bypass, replica_groups=rg,
                            ins=[mneg_loc[:]], outs=[mneg_full[:]])
                    else:
                        nc.sync.dma_start(out=mneg_full[:], in_=mneg_loc[:])

                    # ---------------- PASS 2 ----------------
                    # q = exp(msg - m_b); s = sum_k q  (cache keeps msg)
                    s_sh = statsp.tile([P, T * H], F32, name=f"st_{il}b", tag="stats")
                    rs_full = dramp.tile([N, H], F32, name=f"rs_full_{il}",
                                         addr_space=shared_as if R > 1 else "Local")
                    q_dram = dramp.tile([T * P, FB], F32, name=f"q_{il}")
                    for t in range(T):
                        ct = cache[:, t * FB:(t + 1) * FB]
                        g = gbufp.tile([P, FB], F32, name=f"g2_{il}_{t}", tag="gbuf")
                        nc.scalar.copy(g[:], ct)
                        nc.gpsimd.indirect_dma_start(
                            out=g[:], out_offset=None, in_=mneg_full[:],
                            in_offset=IndirectOffsetOnAxis(
                                ap=bidx_s[:, (l * T + t) * K:(l * T + t + 1) * K],
                                axis=0),
                            compute_op=ALU.add)
                        nc.scalar.activation(g[:], g[:], AF.Exp)
                        # stash q for pass 3 (sequential DMA, idle Sync queue)
                        nc.sync.dma_start(out=q_dram[t * P:(t + 1) * P, :], in_=g[:])
                        nc.vector.tensor_reduce(
                            out=s_sh[:, t * H:(t + 1) * H],
                            in_=g[:].rearrange("p (k h) -> p h k", k=K),
                            axis=mybir.AxisListType.X, op=ALU.add)


                    # rs = 1 / (s + 1e-16)
                    nc.vector.tensor_scalar(
                        out=s_sh[:], in0=s_sh[:], scalar1=1e-16, scalar2=None,
                        op0=ALU.add)
                    nc.vector.reciprocal(s_sh[:], s_sh[:])
                    rs_loc = dramp.tile([SH, H], F32, name=f"rs_loc_{il}")
                    nc.sync.dma_start(
                        out=rs_loc[:].rearrange("(t p) h -> p t h", p=P),
                        in_=s_sh[:].rearrange("p (t h) -> p t h", t=T))
                    if R > 1:
                        nc.gpsimd.collective_compute(
                            "AllGather", ALU.bypass, replica_groups=rg,
                            ins=[rs_loc[:]], outs=[rs_full[:]])
                    else:
                        nc.sync.dma_start(out=rs_full[:], in_=rs_loc[:])

                # ---------------- PASS 3 + MLP ----------------
                aggT = aggp.tile([P, SH], F32, name=f"aggT_{il}", tag="aggT")
                with tc.tile_pool(name=f"pst_{il}", bufs=2, space="PSUM") as pstp:
                    # per-layer-link weight slices
                    w1T_s = bigcp.tile([P, C2], F32, name=f"w1Ts_{il}", tag="w1T")
                    nc.sync.dma_start(out=w1T_s[:],
                                      in_=w1T_d[:, il * C2:(il + 1) * C2])
                    w2T_s = bigcp.tile([P, 2 * H], F32, name=f"w2Ts_{il}", tag="w2T")
                    nc.sync.dma_start(out=w2T_s[:],
                                      in_=w2T_d[:, il * 2 * H:(il + 1) * 2 * H])
                    for t in range(T):
                        ct = cache[:, t * FB:(t + 1) * FB]
                        # reload q stashed by pass 2 (bitwise identical)
                        g1 = gbufp.tile([P, FB], F32, name=f"g3a_{il}_{t}",
                                        tag="gbuf")
                        nc.sync.dma_start(out=g1[:],
                                          in_=q_dram[t * P:(t + 1) * P, :])
                        g2 = gbufp.tile([P, FB], F32, name=f"g3b_{il}_{t}",
                                        tag="gbuf")
                        nc.gpsimd.indirect_dma_start(
                            out=g2[:], out_offset=None, in_=rs_full[:],
                            in_offset=IndirectOffsetOnAxis(
                                ap=bidx_s[:, (l * T + t) * K:(l * T + t + 1) * K],
                                axis=0))
                        # alpha = q * rs_b ; w = msg * alpha  (reference order)
                        nc.vector.tensor_tensor(out=g2[:], in0=g1[:], in1=g2[:],
                                                op=ALU.mult)
                        nc.vector.tensor_tensor(out=g2[:], in0=ct, in1=g2[:],
                                                op=ALU.mult)
                        ast = smallp.tile([P, H], F32, name=f"ast_{il}_{t}", tag="ast")
                        nc.vector.tensor_reduce(
                            out=ast[:], in_=g2[:].rearrange("p (k h) -> p h k", k=K),
                            axis=mybir.AxisListType.X, op=ALU.add)
                        pt = pstp.tile([P, P], F32, name=f"pt_{il}_{t}", tag="pt")
                        nc.tensor.transpose(pt[:], ast[:], ident[:])
                        nc.scalar.copy(aggT[:, t * H:(t + 1) * H], pt[:])

                    # out = agg + dst (explicit IEEE add, reference order)
                    if i > 0:
                        nc.vector.tensor_tensor(out=aggT[:], in0=aggT[:],
                                                in1=h_T_prev[:], op=ALU.add)

                    with tc.tile_pool(name=f"psm_{il}", bufs=2, space="PSUM") as psmp:
                        for c in range(NCH):
                            sl = slice(c * CH, (c + 1) * CH)
                            h1 = [None, None]
                            for half in range(2):
                                hp = psmp.tile([P, CH], F32,
                                               name=f"h1_{il}_{c}_{half}", tag="h1")
                                w1sl = w1T_s[:, half * H:half * H + H]
                                if i == 0:
                                    nc.tensor.matmul(hp[:], lhsT=w1sl,
                                                     rhs=aggT[:, sl],
                                                     start=True, stop=False)
                                    dd = bigcp.tile([P, CH], F32,
                                                    name=f"d1_{il}_{c}_{half}",
                                                    tag="d1")
                                    nc.sync.dma_start(
                                        out=dd[:],
                                        in_=d1T_d[l * C2 + half * H:
                                                  l * C2 + half * H + H, sl])
                                    nc.tensor.matmul(hp[:], lhsT=ident[:], rhs=dd[:],
                                                     start=False, stop=True)
                                else:
                                    nc.tensor.matmul(hp[:], lhsT=w1sl,
                                                     rhs=aggT[:, sl],
                                                     start=True, stop=True)
                                h1[half] = hp
                            for half in range(2):
                                # BN (x*A+B) then relu=max(.,0) on DVE: exact
                                # IEEE mult/add + NaN-killing max (matches XLA)
                                r1 = smallp.tile([P, CH], F32,
                                                 name=f"r1_{il}_{c}_{half}", tag="r1")
                                nc.vector.tensor_scalar(
                                    out=r1[:], in0=h1[half][:],
                                    scalar1=bnA_s[:, il * 2 + half:il * 2 + half + 1],
                                    scalar2=bnB_s[:, il * 2 + half:il * 2 + half + 1],
                                    op0=ALU.mult, op1=ALU.add)
                                nc.vector.tensor_scalar(
                                    out=r1[:], in0=r1[:], scalar1=0.0, scalar2=None,
                                    op0=ALU.max)
                                w2sl = w2T_s[:, half * H:half * H + H]
                                nc.tensor.matmul(
                                    y_ps[:, sl], lhsT=w2sl, rhs=r1[:],
                                    start=(l == 0 and half == 0),
                                    stop=(l == L - 1 and half == 1),
                                    skip_group_check=True)

            # ---- finalize layer: h_T = leaky_relu(y) (or y for last layer) ----
            h_T = htp.tile([P, SH], F32, name=f"hT_{i}", tag="hT")
            if i < NL - 1:
                # leaky_relu(y, 0.01) = max(y, 0.01*y); tmp borrows dead cache space
                ytmp = cache[:, 0:SH]
                nc.vector.tensor_scalar(
                    out=ytmp, in0=y_ps[:], scalar1=0.01, scalar2=None, op0=ALU.mult)
                nc.vector.tensor_tensor(out=h_T[:], in0=y_ps[:], in1=ytmp,
                                        op=ALU.max)
            else:
                nc.scalar.copy(h_T[:], y_ps[:])

            # node-major write-out (+ all-gather for next layer, per half)
            with tc.tile_pool(name=f"psf_{i}", bufs=2, space="PSUM") as psfp:
                if i < NL - 1:
                    h_loc = dramp.tile([SH, H], F32, name=f"h_loc_{i}")
                    dst_dram = h_loc
                else:
                    dst_dram = None
                for t in range(T):
                    pt = psfp.tile([P, P], F32, name=f"ptf_{i}_{t}", tag="ptf")
                    nc.tensor.transpose(pt[:], h_T[:, t * H:(t + 1) * H], ident[:])
                    nm = smallp.tile([P, H], F32, name=f"nm_{i}_{t}", tag="nm")
                    nc.scalar.copy(nm[:], pt[:])
                    if dst_dram is not None:
                        nc.sync.dma_start(out=dst_dram[t * P:(t + 1) * P, :],
                                          in_=nm[:])
                    else:
                        nc.sync.dma_start(out=out_d[t * P:(t + 1) * P, :], in_=nm[:])
                if i < NL - 1:
                    if R > 1:
                        h_full = dramp.tile([N, H], F32, name=f"h_full_{i}",
                                            addr_space=shared_as)
                        nc.gpsimd.collective_compute(
                            "AllGather", ALU.bypass, replica_groups=rg,
                            ins=[h_loc[:]], outs=[h_full[:]])
                    else:
                        h_full = h_loc
                    h_full_prev = h_full
            h_T_prev = h_T


# ---------------------------------------------------------------------------
# Host-side preprocessing
# ---------------------------------------------------------------------------

def prep_inputs(inputs, cfg):
    """Full numpy inputs -> list of per-core in_maps."""
    N, K, L, NL, H, R = cfg["N"], cfg["K"], cfg["L"], cfg["NL"], cfg["H"], cfg["R"]
    CIN = cfg["CIN"]
    SH, T, FB, C2, CH, NCH = _derived(cfg)
    EL = N * K

    x = np.asarray(inputs["x"], np.float32)
    ei = np.asarray(inputs["ei_flat"]).astype(np.int64)
    ea = np.asarray(inputs["ea_flat"], np.float32)
    nbr = np.asarray(inputs["nbr_flat"]).astype(np.int64)
    w_src0 = np.asarray(inputs["w_src0"], np.float32)
    w_dst0 = np.asarray(inputs["w_dst0"], np.float32)
    w_edge = np.asarray(inputs["w_edge"], np.float32)
    w1 = np.asarray(inputs["w1"], np.float32)
    bn_g = np.asarray(inputs["bn_g"], np.float32)
    bn_b = np.asarray(inputs["bn_b"], np.float32)
    bn_m = np.asarray(inputs["bn_m"], np.float32)
    bn_v = np.asarray(inputs["bn_v"], np.float32)
    w2 = np.asarray(inputs["w2"], np.float32)

    assert (nbr >= 0).all() and (nbr < EL).all(), "padded/oob nbr not supported"

    # composed per-(node, k) indices/values, per link
    a_l, b_l, c_l = [], [], []
    for l in range(L):
        j = nbr[:, l * K:(l + 1) * K]                      # [N, K] edge ids
        ei_l = ei[:, l * EL:(l + 1) * EL]
        a_l.append(ei_l[0][j])                             # [N, K] src node
        b_l.append(ei_l[1][j])                             # [N, K] dst node
        c_l.append(ea[l * EL:(l + 1) * EL, 0][j])          # [N, K] edge attr

    # layer-0 host folds
    src0 = np.concatenate(
        [(x @ w_src0[l].T) for l in range(L)], axis=0).astype(np.float32)  # [L*N,H]
    d1 = [
        (x @ (w1[0, l] @ w_dst0[l]).T).astype(np.float32)  # [N, 2H]
        for l in range(L)
    ]

    # weight repacks (shared across cores)
    web = np.zeros((P, NL * L * H), np.float32)
    w1T = np.zeros((P, NL * L * C2), np.float32)
    w2T = np.zeros((P, NL * L * 2 * H), np.float32)
    bnA = np.zeros((P, NL * L * 2), np.float32)
    bnB = np.zeros((P, NL * L * 2), np.float32)
    for i in range(NL):
        for l in range(L):
            il = i * L + l
            web[:, il * H:(il + 1) * H] = w_edge[i, l, :, 0][None, :]
            w1T[:, il * C2:(il + 1) * C2] = w1[i, l].T  # [H, 2H]
            A = (bn_g[i, l] / np.sqrt(bn_v[i, l] + 1e-5)).astype(np.float32)
            B = (bn_b[i, l] - bn_m[i, l] * A).astype(np.float32)
            for half in range(2):
                w2T[:, (il * 2 + half) * H:(il * 2 + half + 1) * H] = \
                    w2[i, l][:, half * P:(half + 1) * P].T  # [128(c2), H]
                bnA[:, il * 2 + half] = A[half * P:(half + 1) * P]
                bnB[:, il * 2 + half] = B[half * P:(half + 1) * P]

    def shard_ct(arr, r):
        # [N, K] -> [K, T*P] with [k, t*P+p] = arr[r*SH+t*P+p, k]
        s = arr[r * SH:(r + 1) * SH].reshape(T, P, K)
        return np.ascontiguousarray(
            s.transpose(2, 0, 1).reshape(K, T * P).astype(np.float32))

    def shard_pk(arr, r, dtype):
        # [N, K] -> rows of core r -> [P, T*K] with [p, t*K+k] = arr[r*SH+t*P+p, k]
        s = arr[r * SH:(r + 1) * SH].reshape(T, P, K).transpose(1, 0, 2)
        return np.ascontiguousarray(s.reshape(P, T * K).astype(dtype))

    in_maps = []
    for r in range(R):
        aidx = np.concatenate([shard_pk(a_l[l], r, np.int32) for l in range(L)], 1)
        bidx = np.concatenate([shard_pk(b_l[l], r, np.int32) for l in range(L)], 1)
        cnm = np.concatenate([shard_pk(c_l[l], r, np.float32) for l in range(L)], 1)
        d1T = np.concatenate(
            [np.ascontiguousarray(d1[l][r * SH:(r + 1) * SH].T) for l in range(L)],
            axis=0)  # [L*2H, SH]
        in_maps.append(dict(
            src0=src0, d1T=d1T, aidx=aidx, bidx=bidx, cnm=cnm,
            web=web, w1T=w1T, w2T=w2T, bnA=bnA, bnB=bnB,
        ))
    return in_maps


# ---------------------------------------------------------------------------
# Entry point
# ---------------------------------------------------------------------------

_PROGRAM_CACHE = {}


def _ensure_ntff_hook():
    """Install an antenv.axon_hooks shim (trimmed container lacks it)."""
    import sys
    import types

    try:
        from antenv.axon_hooks import get_axon_ntff_profile_hook  # noqa: F401
        return
    except ImportError:
        pass
    hook = None
    try:
        from trn_agent_boot.trn_boot import _ntff_profile_via_ctypes
        hook = _ntff_profile_via_ctypes("/opt/axon/libaxon_pjrt.so")
    except Exception:
        hook = None
    import antenv
    mod = types.ModuleType("antenv.axon_hooks")
    mod._hook = hook
    mod.get_axon_ntff_profile_hook = lambda: mod._hook
    mod.set_axon_ntff_profile_hook = lambda h: setattr(mod, "_hook", h)
    antenv.axon_hooks = mod
    sys.modules["antenv.axon_hooks"] = mod


def run(inputs, trace=False):
    """Run the full-size kernel; returns (out [N,H], BassKernelResults)."""
    import concourse.bass_utils as bu
    from concourse.bass_utils import run_bass_kernel_spmd

    if trace:
        _ensure_ntff_hook()
        # artifact upload needs fish/coo creds the sandbox lacks
        bu.upload_artifacts = lambda tmpdir: "local://" + tmpdir

    cfg = full_cfg()
    key = "full"
    if key not in _PROGRAM_CACHE:
        _PROGRAM_CACHE[key] = build_program(cfg)
    nc = _PROGRAM_CACHE[key]

    in_maps = prep_inputs(inputs, cfg)
    res = run_bass_kernel_spmd(
        nc, in_maps, core_ids=list(range(cfg["R"])), trace=trace,
    )
    out = np.concatenate([res.results[r]["out"] for r in range(cfg["R"])], axis=0)
    return out.astype(np.float32), res


def kernel(**inputs) -> np.ndarray:
    out, _ = run(inputs, trace=bool(int(os.environ.get("GNN_TRACE", "0"))))
    return out


# revision 28
# speedup vs baseline: 1.0778x; 1.0778x over previous
# GNN message-passing (GENConv-style, 3 layers x 2 link types) on 8 TRN2 cores.
#
# Strategy (node-sharded, gather-from-replicated-tables):
#   - Each core owns a contiguous shard of SH = N/R nodes.
#   - Host precomposes per-(node,k) indices: a = ei0[nbr], b = ei1[nbr], c = ea[nbr].
#   - Per layer-link, three gather passes over the shard's [SH, K, H] workload:
#       P1: gather src rows by a, msg = relu(src_a + c*we); m = max_k msg  (cache msg)
#       P2: gather (-m) rows by b, q = exp(msg - m_b); s = sum_k q; u = msg*q (cache u)
#       P3: gather (1/s) rows by b, agg = sum_k u * rs_b
#     Tables (src=h, -m, 1/(s+1e-16)) live node-major in DRAM; shards are
#     all-gathered across the 8 cores between passes.
#   - MLP tail per link runs transposed on PE: h1_T = w1T.T@(aggT + dstT),
#     BN+relu fused into one ACT op (per-partition scale/bias), y += w2T.T@relu1.
#   - Layer-0 input projections (x@wsT / x@wdT@w1T) are host-folded into the
#     src0 gather table and a d1T additive term.
#
# The kernel() entry point takes the FULL unsharded inputs (as produced by
# reference.setup_inputs) and returns the FULL [N, H] output.

import os
from contextlib import ExitStack

import numpy as np

import concourse.bass as bass
import concourse.mybir as mybir
import concourse.tile as tile
from concourse import bacc
from concourse.bass import IndirectOffsetOnAxis
from concourse.masks import make_identity

F32 = mybir.dt.float32
I32 = mybir.dt.int32
AF = mybir.ActivationFunctionType
ALU = mybir.AluOpType

P = 128  # partitions


def full_cfg():
    return dict(N=16384, K=16, L=2, NL=3, CIN=170, H=128, R=8)


def _derived(cfg):
    N, K, L, NL, H, R = cfg["N"], cfg["K"], cfg["L"], cfg["NL"], cfg["H"], cfg["R"]
    SH = N // R
    assert SH % P == 0
    T = SH // P
    FB = K * H  # free width of one node-tile in the cache
    C2 = 2 * H
    CH = 512  # matmul free-dim chunk
    assert SH % CH == 0
    NCH = SH // CH
    return SH, T, FB, C2, CH, NCH


def build_program(cfg, nc=None):
    """Builds the SPMD per-core Bass program standalone. Returns nc."""
    N, K, L, NL, H, R = cfg["N"], cfg["K"], cfg["L"], cfg["NL"], cfg["H"], cfg["R"]
    SH, T, FB, C2, CH, NCH = _derived(cfg)

    if nc is None:
        nc = bacc.Bacc("TRN2", num_devices=R, debug=False)

    ins = dict(
        src0=nc.dram_tensor("src0", [L * N, H], F32, kind="ExternalInput").ap(),
        d1T=nc.dram_tensor("d1T", [L * C2, SH], F32, kind="ExternalInput").ap(),
        aidx=nc.dram_tensor("aidx", [P, L * T * K], I32, kind="ExternalInput").ap(),
        bidx=nc.dram_tensor("bidx", [P, L * T * K], I32, kind="ExternalInput").ap(),
        cnm=nc.dram_tensor("cnm", [P, L * T * K], F32, kind="ExternalInput").ap(),
        web=nc.dram_tensor("web", [P, NL * L * H], F32, kind="ExternalInput").ap(),
        w1T=nc.dram_tensor("w1T", [P, NL * L * C2], F32, kind="ExternalInput").ap(),
        w2T=nc.dram_tensor("w2T", [P, NL * L * 2 * H], F32,
                           kind="ExternalInput").ap(),
        bnA=nc.dram_tensor("bnA", [P, NL * L * 2], F32, kind="ExternalInput").ap(),
        bnB=nc.dram_tensor("bnB", [P, NL * L * 2], F32, kind="ExternalInput").ap(),
    )
    outs = dict(out=nc.dram_tensor("out", [SH, H], F32, kind="ExternalOutput").ap())

    with tile.TileContext(nc) as tc:
        build_body(tc, outs, ins, cfg)
    if isinstance(nc, bacc.Bacc):
        nc.compile()
    return nc


def build_body(tc, outs, ins, cfg):
    """Emit the kernel body into TileContext tc. outs/ins: dicts of DRAM APs."""
    nc = tc.nc
    N, K, L, NL, H, R = cfg["N"], cfg["K"], cfg["L"], cfg["NL"], cfg["H"], cfg["R"]
    SH, T, FB, C2, CH, NCH = _derived(cfg)

    src0_d = ins["src0"]
    d1T_d = ins["d1T"]
    aidx_d = ins["aidx"]
    bidx_d = ins["bidx"]
    cnm_d = ins["cnm"]
    web_d = ins["web"]
    w1T_d = ins["w1T"]
    w2T_d = ins["w2T"]
    bnA_d = ins["bnA"]
    bnB_d = ins["bnB"]
    out_d = outs["out"]

    rg = [list(range(R))]
    assert T % 2 == 0
    HT = T // 2  # tiles per half-shard (AG granularity)
    # Shared collective outputs need >4 cores; fall back to Local for small sims
    shared_as = "Shared" if R > 4 else "Local"

    with ExitStack() as ctx:
        constp = ctx.enter_context(tc.tile_pool(name="constp", bufs=1))
        cachep = ctx.enter_context(tc.tile_pool(name="cachep", bufs=1))
        gbufp = ctx.enter_context(tc.tile_pool(name="gbufp", bufs=4))
        statsp = ctx.enter_context(tc.tile_pool(name="statsp", bufs=1))
        smallp = ctx.enter_context(tc.tile_pool(name="smallp", bufs=2))
        aggp = ctx.enter_context(tc.tile_pool(name="aggp", bufs=1))
        htp = ctx.enter_context(tc.tile_pool(name="htp", bufs=1))
        ysp = ctx.enter_context(tc.tile_pool(name="ysp", bufs=1, space="PSUM"))
        bigcp = ctx.enter_context(tc.tile_pool(name="bigcp", bufs=1))
        dramp = ctx.enter_context(tc.tile_pool(name="dramp", bufs=1, space="DRAM"))

        # ---- persistent constants ----
        ident = constp.tile([P, P], F32, name="ident")
        make_identity(nc, ident[:])
        bnA_s = constp.tile([P, NL * L * 2], F32, name="bnA_s")
        nc.sync.dma_start(out=bnA_s[:], in_=bnA_d)
        bnB_s = constp.tile([P, NL * L * 2], F32, name="bnB_s")
        nc.sync.dma_start(out=bnB_s[:], in_=bnB_d)
        web_s = constp.tile([P, NL * L * H], F32, name="web_s")
        nc.sync.dma_start(out=web_s[:], in_=web_d)
        cnm_s = constp.tile([P, L * T * K], F32, name="cnm_s")
        nc.sync.dma_start(out=cnm_s[:], in_=cnm_d)
        aidx_s = constp.tile([P, L * T * K], I32, name="aidx_s")
        nc.sync.dma_start(out=aidx_s[:], in_=aidx_d)
        bidx_s = constp.tile([P, L * T * K], I32, name="bidx_s")
        nc.sync.dma_start(out=bidx_s[:], in_=bidx_d)

        cache = cachep.tile([P, T * FB], F32, name="cache")

        h_T_prev = None
        h_full_prev = None

        for i in range(NL):
            y_ps = ysp.tile([P, SH], F32, name=f"y_{i}", tag="y")
            for l in range(L):
                il = i * L + l
                if i == 0:
                    src_tab = src0_d
                    src_elem_off = l * N * H
                else:
                    src_tab = h_full_prev[:]
                    src_elem_off = 0

                # ---------------- PASS 1 ----------------
                # cache_t = c*we; gather-ADD src rows (CCE, exact IEEE add);
                # msg = max(.,0)+1e-7 (DVE max kills NaN like XLA's maximum);
                # mneg = -max_k msg
                mneg_sh = statsp.tile([P, T * H], F32, name=f"st_{il}a", tag="stats")
                mneg_full = dramp.tile([N, H], F32, name=f"mneg_full_{il}",
                                       addr_space=shared_as if R > 1 else "Local")
                if True:
                    for t in range(T):
                        ct = cache[:, t * FB:(t + 1) * FB]
                        # cwe = c[:,k] * we  (ACT broadcast-mul, per-k)
                        for k in range(K):
                            col = (l * T + t) * K + k
                            nc.scalar.mul(
                                ct[:, k * H:(k + 1) * H],
                                web_s[:, il * H:(il + 1) * H],
                                mul=cnm_s[:, col:col + 1],
                            )
                        nc.gpsimd.indirect_dma_start(
                            out=ct,
                            out_offset=None,
                            in_=src_tab,
                            in_offset=IndirectOffsetOnAxis(
                                ap=aidx_s[:, (l * T + t) * K:(l * T + t + 1) * K],
                                axis=0,
                            ),
                            element_offset=src_elem_off,
                            compute_op=ALU.add,
                        )
                        nc.vector.tensor_scalar(
                            out=ct, in0=ct, scalar1=0.0, scalar2=1e-7,
                            op0=ALU.max, op1=ALU.add)
                        nc.vector.tensor_reduce(
                            out=mneg_sh[:, t * H:(t + 1) * H],
                            in_=ct.rearrange("p (k h) -> p h k", k=K),
                            axis=mybir.AxisListType.X, op=ALU.max, negate=True)

                    mneg_loc = dramp.tile([SH, H], F32, name=f"mneg_loc_{il}")
                    nc.sync.dma_start(
                        out=mneg_loc[:].rearrange("(t p) h -> p t h", p=P),
                        in_=mneg_sh[:].rearrange("p (t h) -> p t h", t=T))
                    if R > 1:
                        nc.gpsimd.collective_compute(
                            "AllGather", ALU.bypass, replica_groups=rg,
                            ins=[mneg_loc[:]], outs=[mneg_full[:]])
                    else:
                        nc.sync.dma_start(out=mneg_full[:], in_=mneg_loc[:])

                    # ---------------- PASS 2 ----------------
                    # q = exp(msg - m_b); s = sum_k q  (cache keeps msg)
                    s_sh = statsp.tile([P, T * H], F32, name=f"st_{il}b", tag="stats")
                    rs_full = dramp.tile([N, H], F32, name=f"rs_full_{il}",
                                         addr_space=shared_as if R > 1 else "Local")
                    for t in range(T):
                        ct = cache[:, t * FB:(t + 1) * FB]
                        g = gbufp.tile([P, FB], F32, name=f"g2_{il}_{t}", tag="gbuf")
                        nc.scalar.copy(g[:], ct)
                        nc.gpsimd.indirect_dma_start(
                            out=g[:], out_offset=None, in_=mneg_full[:],
                            in_offset=IndirectOffsetOnAxis(
                                ap=bidx_s[:, (l * T + t) * K:(l * T + t + 1) * K],
                                axis=0),
                            compute_op=ALU.add)
                        nc.scalar.activation(g[:], g[:], AF.Exp)
                        nc.vector.tensor_reduce(
                            out=s_sh[:, t * H:(t + 1) * H],
                            in_=g[:].rearrange("p (k h) -> p h k", k=K),
                            axis=mybir.AxisListType.X, op=ALU.add)


                    # rs = 1 / (s + 1e-16)
                    nc.vector.tensor_scalar(
                        out=s_sh[:], in0=s_sh[:], scalar1=1e-16, scalar2=None,
                        op0=ALU.add)
                    nc.vector.reciprocal(s_sh[:], s_sh[:])
                    rs_loc = dramp.tile([SH, H], F32, name=f"rs_loc_{il}")
                    nc.sync.dma_start(
                        out=rs_loc[:].rearrange("(t p) h -> p t h", p=P),
                        in_=s_sh[:].rearrange("p (t h) -> p t h", t=T))
                    if R > 1:
                        nc.gpsimd.collective_compute(
                            "AllGather", ALU.bypass, replica_groups=rg,
                            ins=[rs_loc[:]], outs=[rs_full[:]])
                    else:
                        nc.sync.dma_start(out=rs_full[:], in_=rs_loc[:])

                # ---------------- PASS 3 + MLP ----------------
                aggT = aggp.tile([P, SH], F32, name=f"aggT_{il}", tag="aggT")
                with tc.tile_pool(name=f"pst_{il}", bufs=2, space="PSUM") as pstp:
                    # per-layer-link weight slices
                    w1T_s = bigcp.tile([P, C2], F32, name=f"w1Ts_{il}", tag="w1T")
                    nc.sync.dma_start(out=w1T_s[:],
                                      in_=w1T_d[:, il * C2:(il + 1) * C2])
                    w2T_s = bigcp.tile([P, 2 * H], F32, name=f"w2Ts_{il}", tag="w2T")
                    nc.sync.dma_start(out=w2T_s[:],
                                      in_=w2T_d[:, il * 2 * H:(il + 1) * 2 * H])
                    for t in range(T):
                        ct = cache[:, t * FB:(t + 1) * FB]
                        # q = exp(msg + mneg_b) recomputed (reference op order)
                        g1 = gbufp.tile([P, FB], F32, name=f"g3a_{il}_{t}",
                                        tag="gbuf")
                        nc.scalar.copy(g1[:], ct)
                        nc.gpsimd.indirect_dma_start(
                            out=g1[:], out_offset=None, in_=mneg_full[:],
                            in_offset=IndirectOffsetOnAxis(
                                ap=bidx_s[:, (l * T + t) * K:(l * T + t + 1) * K],
                                axis=0),
                            compute_op=ALU.add)
                        nc.scalar.activation(g1[:], g1[:], AF.Exp)
                        g2 = gbufp.tile([P, FB], F32, name=f"g3b_{il}_{t}",
                                        tag="gbuf")
                        nc.gpsimd.indirect_dma_start(
                            out=g2[:], out_offset=None, in_=rs_full[:],
                            in_offset=IndirectOffsetOnAxis(
                                ap=bidx_s[:, (l * T + t) * K:(l * T + t + 1) * K],
                                axis=0))
                        # alpha = q * rs_b ; w = msg * alpha  (reference order)
                        nc.vector.tensor_tensor(out=g2[:], in0=g1[:], in1=g2[:],
                                                op=ALU.mult)
                        nc.vector.tensor_tensor(out=g2[:], in0=ct, in1=g2[:],
                                                op=ALU.mult)
                        ast = smallp.tile([P, H], F32, name=f"ast_{il}_{t}", tag="ast")
                        nc.vector.tensor_reduce(
                            out=ast[:], in_=g2[:].rearrange("p (k h) -> p h k", k=K),
                            axis=mybir.AxisListType.X, op=ALU.add)
                        pt = pstp.tile([P, P], F32, name=f"pt_{il}_{t}", tag="pt")
                        nc.tensor.transpose(pt[:], ast[:], ident[:])
                        nc.scalar.copy(aggT[:, t * H:(t + 1) * H], pt[:])

                    # out = agg + dst (explicit IEEE add, reference order)
                    if i > 0:
                        nc.vector.tensor_tensor(out=aggT[:], in0=aggT[:],
                                                in1=h_T_prev[:], op=ALU.add)

                    with tc.tile_pool(name=f"psm_{il}", bufs=2, space="PSUM") as psmp:
                        for c in range(NCH):
                            sl = slice(c * CH, (c + 1) * CH)
                            h1 = [None, None]
                            for half in range(2):
                                hp = psmp.tile([P, CH], F32,
                                               name=f"h1_{il}_{c}_{half}", tag="h1")
                                w1sl = w1T_s[:, half * H:half * H + H]
                                if i == 0:
                                    nc.tensor.matmul(hp[:], lhsT=w1sl,
                                                     rhs=aggT[:, sl],
                                                     start=True, stop=False)
                                    dd = bigcp.tile([P, CH], F32,
                                                    name=f"d1_{il}_{c}_{half}",
                                                    tag="d1")
                                    nc.sync.dma_start(
                                        out=dd[:],
                                        in_=d1T_d[l * C2 + half * H:
                                                  l * C2 + half * H + H, sl])
                                    nc.tensor.matmul(hp[:], lhsT=ident[:], rhs=dd[:],
                                                     start=False, stop=True)
                                else:
                                    nc.tensor.matmul(hp[:], lhsT=w1sl,
                                                     rhs=aggT[:, sl],
                                                     start=True, stop=True)
                                h1[half] = hp
                            for half in range(2):
                                # BN (x*A+B) then relu=max(.,0) on DVE: exact
                                # IEEE mult/add + NaN-killing max (matches XLA)
                                r1 = smallp.tile([P, CH], F32,
                                                 name=f"r1_{il}_{c}_{half}", tag="r1")
                                nc.vector.tensor_scalar(
                                    out=r1[:], in0=h1[half][:],
                                    scalar1=bnA_s[:, il * 2 + half:il * 2 + half + 1],
                                    scalar2=bnB_s[:, il * 2 + half:il * 2 + half + 1],
                                    op0=ALU.mult, op1=ALU.add)
                                nc.vector.tensor_scalar(
                                    out=r1[:], in0=r1[:], scalar1=0.0, scalar2=None,
                                    op0=ALU.max)
                                w2sl = w2T_s[:, half * H:half * H + H]
                                nc.tensor.matmul(
                                    y_ps[:, sl], lhsT=w2sl, rhs=r1[:],
                                    start=(l == 0 and half == 0),
                                    stop=(l == L - 1 and half == 1),
                                    skip_group_check=True)

            # ---- finalize layer: h_T = leaky_relu(y) (or y for last layer) ----
            h_T = htp.tile([P, SH], F32, name=f"hT_{i}", tag="hT")
            if i < NL - 1:
                # leaky_relu(y, 0.01) = max(y, 0.01*y); tmp borrows dead cache space
                ytmp = cache[:, 0:SH]
                nc.vector.tensor_scalar(
                    out=ytmp, in0=y_ps[:], scalar1=0.01, scalar2=None, op0=ALU.mult)
                nc.vector.tensor_tensor(out=h_T[:], in0=y_ps[:], in1=ytmp,
                                        op=ALU.max)
            else:
                nc.scalar.copy(h_T[:], y_ps[:])

            # node-major write-out (+ all-gather for next layer, per half)
            with tc.tile_pool(name=f"psf_{i}", bufs=2, space="PSUM") as psfp:
                if i < NL - 1:
                    h_loc = dramp.tile([SH, H], F32, name=f"h_loc_{i}")
                    dst_dram = h_loc
                else:
                    dst_dram = None
                for t in range(T):
                    pt = psfp.tile([P, P], F32, name=f"ptf_{i}_{t}", tag="ptf")
                    nc.tensor.transpose(pt[:], h_T[:, t * H:(t + 1) * H], ident[:])
                    nm = smallp.tile([P, H], F32, name=f"nm_{i}_{t}", tag="nm")
                    nc.scalar.copy(nm[:], pt[:])
                    if dst_dram is not None:
                        nc.sync.dma_start(out=dst_dram[t * P:(t + 1) * P, :],
                                          in_=nm[:])
                    else:
                        nc.sync.dma_start(out=out_d[t * P:(t + 1) * P, :], in_=nm[:])
                if i < NL - 1:
                    if R > 1:
                        h_full = dramp.tile([N, H], F32, name=f"h_full_{i}",
                                            addr_space=shared_as)
                        nc.gpsimd.collective_compute(
                            "AllGather", ALU.bypass, replica_groups=rg,
                            ins=[h_loc[:]], outs=[h_full[:]])
                    else:
                        h_full = h_loc
                    h_full_prev = h_full
            h_T_prev = h_T


# ---------------------------------------------------------------------------
# Host-side preprocessing
# ---------------------------------------------------------------------------

def prep_inputs(inputs, cfg):
    """Full numpy inputs -> list of per-core in_maps."""
    N, K, L, NL, H, R = cfg["N"], cfg["K"], cfg["L"], cfg["NL"], cfg["H"], cfg["R"]
    CIN = cfg["CIN"]
    SH, T, FB, C2, CH, NCH = _derived(cfg)
    EL = N * K

    x = np.asarray(inputs["x"], np.float32)
    ei = np.asarray(inputs["ei_flat"]).astype(np.int64)
    ea = np.asarray(inputs["ea_flat"], np.float32)
    nbr = np.asarray(inputs["nbr_flat"]).astype(np.int64)
    w_src0 = np.asarray(inputs["w_src0"], np.float32)
    w_dst0 = np.asarray(inputs["w_dst0"], np.float32)
    w_edge = np.asarray(inputs["w_edge"], np.float32)
    w1 = np.asarray(inputs["w1"], np.float32)
    bn_g = np.asarray(inputs["bn_g"], np.float32)
    bn_b = np.asarray(inputs["bn_b"], np.float32)
    bn_m = np.asarray(inputs["bn_m"], np.float32)
    bn_v = np.asarray(inputs["bn_v"], np.float32)
    w2 = np.asarray(inputs["w2"], np.float32)

    assert (nbr >= 0).all() and (nbr < EL).all(), "padded/oob nbr not supported"

    # composed per-(node, k) indices/values, per link
    a_l, b_l, c_l = [], [], []
    for l in range(L):
        j = nbr[:, l * K:(l + 1) * K]                      # [N, K] edge ids
        ei_l = ei[:, l * EL:(l + 1) * EL]
        a_l.append(ei_l[0][j])                             # [N, K] src node
        b_l.append(ei_l[1][j])                             # [N, K] dst node
        c_l.append(ea[l * EL:(l + 1) * EL, 0][j])          # [N, K] edge attr

    # layer-0 host folds
    src0 = np.concatenate(
        [(x @ w_src0[l].T) for l in range(L)], axis=0).astype(np.float32)  # [L*N,H]
    d1 = [
        (x @ (w1[0, l] @ w_dst0[l]).T).astype(np.float32)  # [N, 2H]
        for l in range(L)
    ]

    # weight repacks (shared across cores)
    web = np.zeros((P, NL * L * H), np.float32)
    w1T = np.zeros((P, NL * L * C2), np.float32)
    w2T = np.zeros((P, NL * L * 2 * H), np.float32)
    bnA = np.zeros((P, NL * L * 2), np.float32)
    bnB = np.zeros((P, NL * L * 2), np.float32)
    for i in range(NL):
        for l in range(L):
            il = i * L + l
            web[:, il * H:(il + 1) * H] = w_edge[i, l, :, 0][None, :]
            w1T[:, il * C2:(il + 1) * C2] = w1[i, l].T  # [H, 2H]
            A = (bn_g[i, l] / np.sqrt(bn_v[i, l] + 1e-5)).astype(np.float32)
            B = (bn_b[i, l] - bn_m[i, l] * A).astype(np.float32)
            for half in range(2):
                w2T[:, (il * 2 + half) * H:(il * 2 + half + 1) * H] = \
                    w2[i, l][:, half * P:(half + 1) * P].T  # [128(c2), H]
                bnA[:, il * 2 + half] = A[half * P:(half + 1) * P]
                bnB[:, il * 2 + half] = B[half * P:(half + 1) * P]

    def shard_ct(arr, r):
        # [N, K] -> [K, T*P] with [k, t*P+p] = arr[r*SH+t*P+p, k]
        s = arr[r * SH:(r + 1) * SH].reshape(T, P, K)
        return np.ascontiguousarray(
            s.transpose(2, 0, 1).reshape(K, T * P).astype(np.float32))

    def shard_pk(arr, r, dtype):
        # [N, K] -> rows of core r -> [P, T*K] with [p, t*K+k] = arr[r*SH+t*P+p, k]
        s = arr[r * SH:(r + 1) * SH].reshape(T, P, K).transpose(1, 0, 2)
        return np.ascontiguousarray(s.reshape(P, T * K).astype(dtype))

    in_maps = []
    for r in range(R):
        aidx = np.concatenate([shard_pk(a_l[l], r, np.int32) for l in range(L)], 1)
        bidx = np.concatenate([shard_pk(b_l[l], r, np.int32) for l in range(L)], 1)
        cnm = np.concatenate([shard_pk(c_l[l], r, np.float32) for l in range(L)], 1)
        d1T = np.concatenate(
            [np.ascontiguousarray(d1[l][r * SH:(r + 1) * SH].T) for l in range(L)],
            axis=0)  # [L*2H, SH]
        in_maps.append(dict(
            src0=src0, d1T=d1T, aidx=aidx, bidx=bidx, cnm=cnm,
            web=web, w1T=w1T, w2T=w2T, bnA=bnA, bnB=bnB,
        ))
    return in_maps


# ---------------------------------------------------------------------------
# Entry point
# ---------------------------------------------------------------------------

_PROGRAM_CACHE = {}


def _ensure_ntff_hook():
    """Install an antenv.axon_hooks shim (trimmed container lacks it)."""
    import sys
    import types

    try:
        from antenv.axon_hooks import get_axon_ntff_profile_hook  # noqa: F401
        return
    except ImportError:
        pass
    hook = None
    try:
        from trn_agent_boot.trn_boot import _ntff_profile_via_ctypes
        hook = _ntff_profile_via_ctypes("/opt/axon/libaxon_pjrt.so")
    except Exception:
        hook = None
    import antenv
    mod = types.ModuleType("antenv.axon_hooks")
    mod._hook = hook
    mod.get_axon_ntff_profile_hook = lambda: mod._hook
    mod.set_axon_ntff_profile_hook = lambda h: setattr(mod, "_hook", h)
    antenv.axon_hooks = mod
    sys.modules["antenv.axon_hooks"] = mod


def run(inputs, trace=False):
    """Run the full-size kernel; returns (out [N,H], BassKernelResults)."""
    import concourse.bass_utils as bu
    from concourse.bass_utils import run_bass_kernel_spmd

    if trace:
        _ensure_ntff_hook()
        # artifact upload needs fish/coo creds the sandbox lacks
        bu.upload_artifacts = lambda tmpdir: "local://" + tmpdir

    cfg = full_cfg()
    key = "full"
    if key not in _PROGRAM_CACHE:
        _PROGRAM_CACHE[key] = build_program(cfg)
    nc = _PROGRAM_CACHE[key]

    in_maps = prep_inputs(inputs, cfg)
    res = run_bass_kernel_spmd(
        nc, in_maps, core_ids=list(range(cfg["R"])), trace=trace,
    )
    out = np.concatenate([res.results[r]["out"] for r in range(cfg["R"])], axis=0)
    return out.astype(np.float32), res


def kernel(**inputs) -> np.ndarray:
    out, _ = run(inputs, trace=bool(int(os.environ.get("GNN_TRACE", "0"))))
    return out
